# revision 40
# baseline (speedup 1.0000x reference)
"""DefocusLKPN Trainium2 kernel (v2).

Computes, per batch element (reference semantics):
    r      = clip(alpha * defocus + tanh(unet[:,100]), 0, 3)
    disk_k = sigmoid(5*(r - dist_k))            (25 taps, 6 distinct dists)
    w_ck   = exp(l_ck) * disk_k                 (l = unet[:, :100] logits)
    out_c  = sum_k w_ck * patch_ck / sum_k w_ck + x_c

The softmax normalizer and the EPS clamp of the reference cancel exactly.
The alpha * defocus product is folded into the defocus array on the host.

Sharding: pure data parallel, batch 16 -> 2 per core across 8 cores.

Per-core layout: partition dim = H (128); free dim packs (b, w) = 256 for
pixel planes and (k, b, w) for the 25-tap weight planes.

v2 design notes (from HW traces of v1):
  * v1 spent ~50 us of SYNC-engine time pushing HWDGE descriptors (512B
    packets); the engine blocks while its queue drains.  All bulk loads now
    go through the gpsimd SWDGE path: ~1 us fixed engine cost per dma_start,
    async drain, 4KB packets.  Only the small df/u100/sid/x loads and the
    output stores ride the (otherwise idle) sync HWDGE queue.
  * The 5 row-shifted, column-padded x copies are built as in v1: shifted-
    identity matmuls into PSUM (zero-fills edge rows) with ACT copybacks
    woven between c0's exp slices.  (Folding the row shift into the
    accumulate matmul instead is WRONG: it would shift the w factor too.)
    The k-reduction runs as identity-matmul accumulation of the [m | w]
    two-chunk moving APs, banks alternating by tap parity so PSUM
    read-modify-write never stalls the PE.
  * ACT table sets: tanh is computed as 2*sigmoid(2x)-1 (the affine folds
    into the radius clamp and the disk bias), so phase 1 only needs the
    sigmoid table and the rest of the kernel only needs exp+ln
    (natural_log_exp_and_others): 2 table loads instead of 3.
  * Radius chain and the c0-c2 epilogue multiply/adds run on the Pool
    engine's idle slots; DVE keeps only the tap products, s25 build and the
    PSUM-reading combines.  1/den uses ACT Ln then Exp(scale=-1).
"""

import sys

sys.path.insert(0, "/opt/trn_rl_repo")

import numpy as np

import concourse.bass as bass
import concourse.mybir as mybir
from concourse.tile import TileContext
from concourse.bass_utils import run_bass_kernel_spmd

F32 = mybir.dt.float32
FP16 = mybir.dt.float16
AF = mybir.ActivationFunctionType
ALU = mybir.AluOpType

MM_DT = FP16

N_CORES = 8
B, C, H, W = 16, 4, 128, 128
BL = B // N_CORES            # 2 batch elements per core
BLC = BL * C                 # 8 (b, c) blocks
KK = 25
BW = BL * W                  # 256: (b, w) free block
WP = W + 4                   # 132: padded width per (b, c) block
DB = 5 * BW                  # 1280: one dy-group block (5 planes)

# distinct tap distances; k = (dy+2)*5 + (dx+2)
DISTS = [0.0, 1.0, np.sqrt(2.0), 2.0, np.sqrt(5.0), np.sqrt(8.0)]
# (dist_index, base_k, [(step, count), (step2, count2)]): tap sets sharing
# that dist, {base + i*s1 + j*s2}.  Ordered so the dy0 taps (k 0..4) land
# first: the first w-products depend on them.
GROUPS = [
    (5, 0, [(20, 2), (4, 2)]),      # dist sqrt8:  {0, 4, 20, 24}
    (4, 1, [(20, 2), (2, 2)]),      # dist sqrt5:  {1, 3, 21, 23}
    (3, 2, [(12, 2), (8, 2)]),      # dist 2:      {2, 10, 14, 22}
    (4, 5, [(10, 2), (4, 2)]),      # dist sqrt5:  {5, 9, 15, 19}
    (2, 6, [(10, 2), (2, 2)]),      # dist sqrt2:  {6, 8, 16, 18}
    (1, 7, [(6, 2), (4, 2)]),       # dist 1:      {7, 11, 13, 17}
    (0, 12, []),                    # dist 0:      {12}
]

# l-load piece / exp-slice splits per channel: tuple of (k0, nk).  The last
# channel's trailing pieces are small so the load->exp->product->matmul->
# epilogue chain after the final DMA byte is as short as possible.
LOAD_SPLIT = {
    0: ((0, 5), (5, 10), (15, 10)),
    1: ((0, 10), (10, 15)),
    2: ((0, 10), (10, 15)),
    3: ((0, 10), (10, 10), (20, 3), (23, 2)),
}
EXP_SPLIT = {
    0: ((0, 5), (5, 5), (10, 5), (15, 5), (20, 5)),
    1: ((0, 10), (10, 10), (20, 5)),
    2: ((0, 10), (10, 10), (20, 5)),
    3: ((0, 10), (10, 10), (20, 3), (23, 2)),
}
# per-dy tap splits for the products/matmuls (last channel splits dy4)
DY_SPLIT = {dy: ((0, 5),) for dy in range(5)}
DY_SPLIT_LAST = {**DY_SPLIT, 4: ((0, 3), (3, 2))}
# after which c0 exp slice each xs copyback runs (dyi -> slice index)
XS_AFTER = {0: 0, 1: 1, 3: 3, 4: 4}


def _split_wide_waits(nc, max_waits=1):
    """The walrus build here accepts at most one semaphore wait per
    instruction; move extra waits onto preceding Drains on the same engine."""
    n = 0
    for func in nc.m.functions:
        for bb in func.blocks:
            out = []
            changed = False
            for ins in bb.instructions:
                si = ins.sync_info
                if si is not None and si.on_wait and len(si.on_wait) > max_waits:
                    waits = list(si.on_wait)
                    keep, rest = waits[:max_waits], waits[max_waits:]
                    for i in range(0, len(rest), max_waits):
                        n += 1
                        out.append(
                            mybir.InstDrain(
                                name=f"splitwait-{n}",
                                opcode="Drain",
                                engine=ins.engine,
                                sync_info=mybir.SyncInfo(
                                    on_wait=list(rest[i : i + max_waits]),
                                    on_update=[],
                                ),
                            )
                        )
                    si.on_wait = keep
                    changed = True
                out.append(ins)
            if changed:
                bb.instructions = out
    return n


def _ap(t, extra_off, dims):
    """AP over tile `t` keeping its partition dim, with free dims
    [[step, count], ...] in elements and an extra element offset."""
    return bass.AP(t.tensor, t.offset + extra_off, [list(t.ap[0])] + [list(d) for d in dims])


def _dedupe_ldweights(nc):
    """Mark matmuls whose stationary operand is identical to the previous
    matmul's on the PE queue as non-self-loading: walrus then skips the
    (otherwise per-matmul) LDWEIGHTS emission.  ~100 of the 108 matmuls here
    share the identity stationary."""
    n = 0
    for func in nc.m.functions:
        for bb in func.blocks:
            prev = None
            for ins in bb.instructions:
                if not isinstance(ins, mybir.InstMatmult):
                    continue
                w = ins.ins[1]
                key = (w.memref, w.offset, str(w.ap))
                if prev is not None and key == prev:
                    ins.ldweights = False
                    n += 1
                prev = key
    return n


# aux tensor: host-packed [17, H, W] f32 planes, loaded in ONE fp16
# cast-DMA at the head of the SWDGE queue: {alpha*defocus (BL), unet[100]
# (BL), shift matrices S_dy0..4 (eye(128, k=2-dy)), x (BL*C)}.
AUX_CH = BL + BL + 5 + BLC     # 17


def _build():
    nc = bass.Bass("TRN2", num_devices=N_CORES)

    auxl = nc.dram_tensor("aux", [AUX_CH, H, W], F32, kind="ExternalInput")
    ul = nc.dram_tensor("unet", [BL, 4 * KK + 1, H, W], F32, kind="ExternalInput")
    yl = nc.dram_tensor("y", [BL, C, H, W], F32, kind="ExternalOutput")

    UCH = ul.shape[1]          # 101
    HWr = H * W                # plane stride in DRAM

    def load_l(eng, l, c, k0, nk):
        # DMA APs balance to at most 3 dims, so one dma_start per b
        for b in range(BL):
            eng.dma_start(
                out=_ap(l, k0 * BW + b * W, [[BW, nk], [1, W]]),
                in_=bass.AP(
                    ul, (c * KK + k0 + b * UCH) * HWr,
                    [[W, H], [HWr, nk], [1, W]],
                ),
            )

    with TileContext(nc) as tc:
        with (
            tc.tile_pool(name="fix", bufs=1) as fix,
            tc.tile_pool(name="lp", bufs=4) as lp,
            tc.tile_pool(name="ep", bufs=2) as ep,
            tc.tile_pool(name="mp", bufs=8) as mp,
            tc.tile_pool(name="op", bufs=3) as op,
            tc.tile_pool(name="ps", bufs=2, space="PSUM") as ps,
            tc.tile_pool(name="psx", bufs=2, space="PSUM") as psx,
        ):
            # ---- ONE prologue cast-load at the FRONT of the gpsimd SWDGE
            # queue: the sync HWDGE queue's 512B packets get ~1/9 of the
            # SDMA round-robin once the 4KB-packet bulk loads start, and
            # each extra dma_start costs ~1.1us of issue serialization, so
            # everything latency-critical rides one fp16 cast-DMA.
            aux = fix.tile([128, AUX_CH * W], MM_DT)
            # two pieces: the 4 radius planes land first (small sem unit so
            # the tanh chain is not gated on the x/sid bytes)
            nc.gpsimd.dma_start(
                out=aux[:, 0 : 4 * W],
                in_=bass.AP(auxl, 0, [[W, H], [HWr, 4], [1, W]]),
            )
            nc.gpsimd.dma_start(
                out=aux[:, 4 * W : AUX_CH * W],
                in_=bass.AP(auxl, 4 * HWr, [[W, H], [HWr, AUX_CH - 4], [1, W]]),
            )
            df = aux[:, 0:BW]
            u100 = aux[:, BW : 2 * BW]
            sid = aux[:, 2 * BW : 2 * BW + 5 * W]
            xb = aux[:, 2 * BW + 5 * W : AUX_CH * W]
            idt = sid[:, 2 * 128 : 3 * 128]

            # ---- memsets on the (early-idle) DVE so the Pool queue is pure
            # DMA issue from the start.  High priority (with the radius
            # chain below) so they stay ahead of the x cast in the DVE
            # queue; the memsets MUST sort before the s25 ops that (via the
            # ACT tanhs) depend on bt, or the in-order queues deadlock.
            with tc.high_priority():
                bt = fix.tile([128, 6], F32)
                for d in range(6):
                    nc.vector.memset(bt[:, d : d + 1], float(-2.5 * DISTS[d]))
                # xs[dyi]: row-shifted (by Dy=dyi-2), column-padded (pad 2)
                # fp16 copies of x; shifts via S_dyi.T @ xb on the PE.
                xs = []
                for dyi in range(5):
                    t = fix.tile([128, BLC * WP], MM_DT, name=f"xs{dyi}")
                    nc.vector.memset(_ap(t, 0, [[WP, BLC], [1, 2]]), 0.0)
                    nc.vector.memset(_ap(t, 2 + W, [[WP, BLC], [1, 2]]), 0.0)
                    xs.append(t)

                # ---- radius chain (same high-priority block) -------------
                # disk = sigmoid(5(r-d)) = 0.5 + 0.5*tanh(2.5(r-d)): tanh
                # lives in the same ACT table set as exp, so the pre-Ln
                # phase needs no table switch; the 0.5-affine folds into the
                # s25 tensor_scalar expansion below.
                dtan = fix.tile([128, BW], MM_DT)
                nc.scalar.activation(dtan[:], u100[:], AF.Tanh)
                r0 = fix.tile([128, BW], MM_DT)
                nc.vector.tensor_tensor(r0[:], df[:], dtan[:], ALU.add)
                rc = fix.tile([128, BW], MM_DT)
                nc.vector.tensor_scalar(rc[:], r0[:], 0.0, 3.0, ALU.max, ALU.min)

                # s6[d] = tanh(2.5*r - 2.5*dist_d), one tile per d so the
                # s25 expansion of a group only waits on its own tanh
                s6 = [fix.tile([128, BW], MM_DT, name=f"s6_{d}") for d in range(6)]
                seen = set()
                for d, _, _ in GROUPS:        # emission order = use order
                    if d in seen:
                        continue
                    seen.add(d)
                    nc.scalar.activation(
                        s6[d][:], rc[:], AF.Tanh,
                        bias=bt[:, d : d + 1], scale=2.5,
                    )
                # s25[k] = 0.5*s6[dist(k)] + 0.5: replicated so the
                # per-(c,dy) w-product is a single contiguous DVE instr.
                s25 = fix.tile([128, KK * BW], MM_DT)
                for d, base, steps in GROUPS:
                    if steps:
                        (s1, c1), (s2, c2) = steps
                        odims = [[s1 * BW, c1], [s2 * BW, c2], [1, BW]]
                        idims = [[0, c1], [0, c2], [1, BW]]
                    else:
                        odims = [[1, BW]]
                        idims = [[1, BW]]
                    nc.vector.tensor_scalar(
                        _ap(s25, base * BW, odims), _ap(s6[d], 0, idims),
                        0.5, 0.5, ALU.mult, ALU.add,
                    )

            # l tiles land as fp16 via the SWDGE cast-during-DMA path: half
            # the SBUF, so all four channels prefetch without ring stalls.
            ls = []
            for c in range(C):
                l = lp.tile([128, KK * BW], MM_DT, name="l")
                ls.append(l)
                for k0, nk in LOAD_SPLIT[c]:
                    load_l(nc.gpsimd, l, c, k0, nk)

            # x center copy into xs[2] and the 4 row-shift matmuls into
            # PSUM (copied back to SBUF by ACT between c0 exps); xb is the
            # fp16 x straight from the aux cast-load
            nc.vector.tensor_copy(
                _ap(xs[2], 2, [[WP, BLC], [1, W]]),
                _ap(xb, 0, [[W, BLC], [1, W]]),
            )
            psts = {}
            for dyi in (0, 1, 3, 4):
                pst = psx.tile([128, BLC * W], F32, name="pshift")
                psts[dyi] = pst
                nc.tensor.matmul(
                    pst[:, 0:512], sid[:, dyi * 128 : (dyi + 1) * 128],
                    xb[:, 0:512], start=True, stop=True,
                )
                nc.tensor.matmul(
                    pst[:, 512:1024], sid[:, dyi * 128 : (dyi + 1) * 128],
                    xb[:, 512:1024], start=True, stop=True,
                )

            # ---- per-channel main loop -----------------------------------
            # nd1/nd2 accumulate [num | den] for alternate taps; the num
            # matmul of tap (dy, j) uses the shifted stationary S_dy, the den
            # matmul the identity.  mdy packs 5 m planes then 5 w planes.
            outs = []
            nds = []
            ndcs = []
            sbbs = []
            rdens = []
            for c in range(C):
                nd1 = ps.tile([128, 2 * BW], F32, name="nd1")
                nd2 = ps.tile([128, 2 * BW], F32, name="nd2")
                nds.append((nd1, nd2))
                l = ls[c]

                lexp = ep.tile([128, KK * BW], MM_DT, name="lexp")
                esl = EXP_SPLIT[c]
                for si, (k0, nk) in enumerate(esl):
                    nc.scalar.activation(
                        lexp[:, k0 * BW : (k0 + nk) * BW],
                        l[:, k0 * BW : (k0 + nk) * BW], AF.Exp,
                    )
                    # c0: weave the xs copybacks between the exp slices
                    if c == 0:
                        for dyi, after in XS_AFTER.items():
                            if after == si:
                                nc.scalar.copy(
                                    _ap(xs[dyi], 2, [[WP, BLC], [1, W]]),
                                    _ap(psts[dyi], 0, [[W, BLC], [1, W]]),
                                )
                    # deferred epilogue pieces for the previous channel ride
                    # the ACT queue between this channel's exp slices
                    if c >= 1 and si == 0:
                        sbb = op.tile([128, 2 * BW], F32, name="sbb")
                        nc.scalar.copy(sbb[:], nds[c - 1][1][:])
                        sbbs.append(sbb)
                dsl = DY_SPLIT_LAST if c == C - 1 else DY_SPLIT
                for dy in range(5):
                    if dy == 2 and c >= 1:
                        ndc = op.tile([128, 2 * BW], F32, name="ndc")
                        nc.vector.tensor_tensor(
                            ndc[:], nds[c - 1][0][:], sbbs[c - 1][:], ALU.add
                        )
                        ndcs.append(ndc)
                    mdy = mp.tile([128, 2 * DB], MM_DT, name="mdy")
                    for j0, nj in dsl[dy]:
                        nb = nj * BW
                        # w block = s25 * lexp (contiguous instr per piece)
                        nc.vector.tensor_tensor(
                            _ap(mdy, DB + j0 * BW, [[1, nb]]),
                            _ap(s25, dy * DB + j0 * BW, [[1, nb]]),
                            _ap(lexp, dy * DB + j0 * BW, [[1, nb]]),
                            ALU.mult,
                        )
                        # m block = w block * xs window (tap j at offset j)
                        nc.vector.tensor_tensor(
                            _ap(mdy, j0 * BW, [[1, nb]]),
                            _ap(mdy, DB + j0 * BW, [[1, nb]]),
                            _ap(xs[dy], c * WP + j0,
                                [[1, nj], [C * WP, BL], [1, W]]),
                            ALU.mult,
                        )
                        # tap j accumulates the two-chunk [m_j | w_j] moving
                        # AP via the identity stationary, banks alternating
                        # by tap parity (PSUM read-modify-write)
                        for j in range(j0, j0 + nj):
                            p = dy * 5 + j
                            tgt = nd1 if p % 2 == 0 else nd2
                            nc.tensor.matmul(
                                tgt[:], idt,
                                _ap(mdy, j * BW, [[DB, 2], [1, BW]]),
                                start=(p < 2), stop=(p >= 23),
                            )
                # previous channel's 1/den on ACT + out on DVE (Pool compute
                # contends with DVE for the shared SBUF port and slows the
                # tap products by ~65%); store two channels late so the sync
                # queue never stalls on compute.
                if c >= 1:
                    rdens.append(_epi_act(nc, op, ndcs[c - 1]))
                    _epi_out(nc.vector, op, outs, rdens[c - 1], ndcs[c - 1], xb, c - 1)
                if c >= 2:
                    nc.sync.dma_start(
                        out=bass.AP(
                            yl, (c - 2) * HWr, [[W, H], [C * HWr, BL], [1, W]]
                        ),
                        in_=outs[c - 2][:],
                    )

            # ---- last channel epilogue (latency critical, DVE) -----------
            sbb = op.tile([128, 2 * BW], F32, name="sbb")
            nc.scalar.copy(sbb[:], nds[C - 1][1][:])
            sbbs.append(sbb)
            ndc = op.tile([128, 2 * BW], F32, name="ndc")
            nc.vector.tensor_tensor(ndc[:], nds[C - 1][0][:], sbbs[C - 1][:], ALU.add)
            ndcs.append(ndc)
            rdens.append(_epi_act(nc, op, ndcs[C - 1]))
            _epi_out(nc.vector, op, outs, rdens[C - 1], ndcs[C - 1], xb, C - 1)
            for c in (C - 2, C - 1):
                nc.sync.dma_start(
                    out=bass.AP(yl, c * HWr, [[W, H], [C * HWr, BL], [1, W]]),
                    in_=outs[c][:],
                )

    _split_wide_waits(nc)
    _dedupe_ldweights(nc)
    return nc


def _epi_act(nc, op, nd):
    """1/den = exp(-ln(den)) on the ACT engine."""
    lden = op.tile([128, BW], F32, name="lden")
    nc.scalar.activation(lden[:], nd[:, BW : 2 * BW], AF.Ln)
    rden = op.tile([128, BW], F32, name="rden")
    nc.scalar.activation(rden[:], lden[:], AF.Exp, scale=-1.0)
    return rden


def _epi_out(eng, op, outs, rden, nd, xb, c):
    """out_c = num * (1/den) + x on the DVE."""
    o1 = op.tile([128, BW], F32, name="o1")
    eng.tensor_tensor(o1[:], nd[:, 0:BW], rden[:], ALU.mult)
    o2 = op.tile([128, BW], F32, name="o2")
    eng.tensor_tensor(
        o2[:], o1[:], _ap(xb, c * W, [[C * W, BL], [1, W]]), ALU.add
    )
    outs.append(o2)


_NC_CACHE = None


def _get_nc():
    global _NC_CACHE
    if _NC_CACHE is None:
        _NC_CACHE = _build()
    return _NC_CACHE


def _make_in_maps(x, defocus_map, unet_out, alpha):
    x = np.asarray(x, dtype=np.float32)
    alpha_s = np.float32(np.asarray(alpha).reshape(-1)[0])
    adf = np.asarray(alpha_s * defocus_map, dtype=np.float32)
    unet_out = np.ascontiguousarray(unet_out, dtype=np.float32)
    # shift matrices S_dyi[q, p] = 1 iff q = p + (dyi-2): accumulate-matmul
    # stationaries (num taps) with zero fill at the edge rows; block dyi=2
    # is the identity (den taps).
    s_np = np.zeros((5, 128, 128), dtype=np.float32)
    for dyi in range(5):
        s_np[dyi] = np.eye(128, k=2 - dyi, dtype=np.float32)
    in_maps = []
    for core in range(N_CORES):
        s = slice(core * BL, (core + 1) * BL)
        aux = np.concatenate(
            [
                adf[s].reshape(BL, H, W),
                unet_out[s, 100],
                s_np,
                x[s].reshape(BLC, H, W),
            ],
            axis=0,
        )
        in_maps.append(
            {
                "aux": np.ascontiguousarray(aux),
                "unet": unet_out[s],
            }
        )
    return in_maps


def run(x, defocus_map, unet_out, alpha, **spmd_kwargs):
    """Run the kernel; returns (output, BassKernelResults)."""
    nc = _get_nc()
    in_maps = _make_in_maps(x, defocus_map, unet_out, alpha)
    res = run_bass_kernel_spmd(nc, in_maps, list(range(N_CORES)), **spmd_kwargs)
    out = np.concatenate([res.results[i]["y"] for i in range(N_CORES)], axis=0)
    return out.astype(np.float32), res


def kernel(x, defocus_map, unet_out, alpha):
    return run(x, defocus_map, unet_out, alpha)[0]
